# revision 7
# baseline (speedup 1.0000x reference)
"""AlignmentEncoder forward on 8 Trainium2 NeuronCores (data-parallel over batch).

Per batch b (one batch per core), with speaker/emotion conditioning folded
into keys/queries host-side:
  keys_enc = conv1d(relu(conv1d(keys, kw1, pad1)), kw2)                 (80, 256)
  queries_enc = conv1d(relu(conv1d(relu(conv1d(q,qw1,pad1)),qw2)),qw3)  (80, 1000)
  a[t,s] = 2*TEMP*qk - TEMP*k2[s]   (the q2 row-constant cancels in both
           log_softmax and softmax and is dropped)
  out_logp = a - ln(sum_s exp a) + log(prior + 1e-8)
  out_attn = e*pm / sum_s(e*pm),  e = exp(a), pm = (prior+1e-8)*(1-mask)

conv1-over-keys runs in fp8e4m3 with DoubleRow packing (weights scaled x16,
un-scaled through kb1*16 / kw2/16); everything else bf16 with f32 PSUM.
Outputs are written bf16 and widened host-side.  Input DMAs are split
across the SP and ACT HWDGE rings so dispatch serialisation doesn't gate
the conv start.
"""

import sys
from contextlib import ExitStack

sys.path.insert(0, "/opt/trn_rl_repo")

import numpy as np
import ml_dtypes

import bass_rust
from bass_rust import add_dep_helper
import concourse.bass as bass
import concourse.mybir as mybir
import concourse.tile as tile

BF16 = ml_dtypes.bfloat16
FP8 = ml_dtypes.float8_e4m3
F32 = np.float32

B, MEL, TXT, ATT, T1, T2 = 8, 80, 512, 80, 1000, 256
TEMP = 0.0005
N_CORES = 8
WS = 16.0  # fp8 weight scale for conv1

_MAX_WAITS = 1  # this walrus build rejects multi-wait instructions


def _split_excess_waits(nc):
    """Move excess sem waits from any instruction onto single-wait NoOps
    inserted immediately before it.  Mid-program instructions get
    same-engine NoOps (program order preserves wait semantics); the
    kernel-tail Drain's waits are spread across all engines — they run in
    parallel and the following all-engine barrier re-synchronises."""
    spread = [
        mybir.EngineType.SP,
        mybir.EngineType.Activation,
        mybir.EngineType.DVE,
        mybir.EngineType.PE,
        mybir.EngineType.Pool,
    ]
    uid = 0
    for blk in nc.m.functions[0].blocks:
        insts = list(blk.instructions)
        out = []
        changed = False
        for inst in insts:
            si = inst.sync_info
            waits = list(si.on_wait) if si is not None and si.on_wait else []
            if len(waits) > _MAX_WAITS:
                si.on_wait = waits[-_MAX_WAITS:]
                extra = waits[: -_MAX_WAITS]
                is_drain = isinstance(inst, mybir.InstDrain)
                for j in range(0, len(extra), _MAX_WAITS):
                    nop = mybir.InstNoOp(name=f"I-waitsplit-{uid}", ins=[], outs=[])
                    nop.engine = (
                        spread[(j // _MAX_WAITS) % len(spread)]
                        if is_drain
                        else inst.engine
                    )
                    uid += 1
                    nop.bass_nofuse = True
                    nop.sync_info = bass_rust.SyncInfo(
                        on_wait=extra[j : j + _MAX_WAITS], on_update=[]
                    )
                    out.append(nop)
                changed = True
            out.append(inst)
        if changed:
            blk.instructions = out


class _TC(tile.TileContext):
    pass


# wblob (bf16) column layout
W_KW2 = 0            # [128, 8*80]
W_QW2A = 640         # [128, 80]
W_QW1 = 720          # [80, 3*160]
W_QW3 = 1200         # [80, 80]
W_QW2B = 1280        # [32, 80]
W_COLS = 1360
# fblob (f32) column layout
F_KB1 = 0            # [128, 8]  (16*kb1)
F_QB1A = 8           # [128, 1]
F_QB1B = 9           # [32, 1]
F_QB2 = 10           # [80, 1]
F_QB3S = 11          # [80, 1]  (2*TEMP*qb3)
F_KB2 = 12           # [80, 1]
F_COLS = 13


def _build_nc(salt=""):
    f32 = mybir.dt.float32
    bf16 = mybir.dt.bfloat16
    fp8 = mybir.dt.float8e4
    AF = mybir.ActivationFunctionType
    OP = mybir.AluOpType
    DR = mybir.MatmulPerfMode.DoubleRow

    nc = bass.Bass("TRN2", target_bir_lowering=False, debug=False, num_devices=N_CORES)
    dp = nc.declare_dram_parameter
    queries_p = dp("queries", [MEL, T1 + 2], bf16, isOutput=False)
    keys_p = dp("keys", [128, 4, T2 + 2], fp8, isOutput=False)
    wblob_p = dp("wblob", [128, W_COLS], bf16, isOutput=False)
    fblob_p = dp("fblob", [128, F_COLS], f32, isOutput=False)
    kw1_p = dp("kw1", [128, 8, 3, 2, 2, 128], fp8, isOutput=False)
    lppm_p = dp("lppm", [128, 2, 8, T2], bf16, isOutput=False)
    out_p = dp("out", [128, 8, 2, T2], bf16, isOutput=True)

    with _TC(nc) as tc, ExitStack() as ctx:
        const = ctx.enter_context(tc.tile_pool(name="const" + salt, bufs=1))
        work = ctx.enter_context(tc.tile_pool(name="work" + salt, bufs=4))
        small = ctx.enter_context(tc.tile_pool(name="small" + salt, bufs=8))

        # ---- t=0 memsets: warm tiles on GpSimd (its stream dispatches
        # ~1us before Vector's), the rest on Vector.
        warm_w = const.tile([128, 8], bf16, tag="warmw")
        nc.gpsimd.memset(warm_w, 0.0)
        warm_rhs = const.tile([128, 512], bf16, tag="warmrhs")
        nc.gpsimd.memset(warm_rhs, 0.0)
        ones80 = const.tile([ATT, 1], bf16, tag="ones80")
        nc.vector.memset(ones80, 1.0)
        ones1 = const.tile([1, 128], bf16, tag="ones1")
        nc.vector.memset(ones1, 1.0)

        # ---- input DMAs.  SP ring: q-chain inputs + lppm; ACT ring: keys
        # + conv1 weights.  Dispatch order = need order; no completion
        # gating (the per-ring FIFO already serialises dispatch).
        qbuf = const.tile([MEL, T1 + 2], bf16, tag="qbuf")
        wblob = const.tile([128, W_COLS], bf16, tag="wblob")
        fblob = const.tile([128, F_COLS], f32, tag="fblob")
        keys_sb = const.tile([128, 4, T2 + 2], fp8, tag="keys")
        kw1_sb = const.tile([128, 8, 3, 2, 2, 128], fp8, tag="kw1")
        lppm = const.tile([128, 2, 8, T2], bf16, tag="lppm")
        nc.sync.dma_start(out=qbuf, in_=queries_p[:])
        nc.scalar.dma_start(out=keys_sb, in_=keys_p[:])
        nc.sync.dma_start(out=wblob, in_=wblob_p[:])
        for g in range(4):
            nc.scalar.dma_start(
                out=kw1_sb[:, 2 * g : 2 * g + 2], in_=kw1_p[:, 2 * g : 2 * g + 2]
            )
        nc.sync.dma_start(out=fblob, in_=fblob_p[:])
        nc.sync.dma_start(out=lppm, in_=lppm_p[:])

        # ACT-table preload: first scalar activation triggers the ~1.3us
        # table load; this runs right after the ACT-ring DMA dispatches,
        # well before the first relu needs it.
        dummy = const.tile([128, 2], f32, tag="dummy")
        nc.vector.memset(dummy, 1.0)
        nc.scalar.activation(dummy[:, 1:2], dummy[:, 0:1], AF.Exp)

        # weight views into the blobs
        kw2_v = wblob[:, W_KW2 : W_KW2 + 640]
        qw2a_v = wblob[:, W_QW2A : W_QW2A + 80]
        qw3_v = wblob[0:MEL, W_QW3 : W_QW3 + 80]
        qw2b_v = wblob[0:32, W_QW2B : W_QW2B + 80]
        kb1_v = fblob[:, F_KB1 : F_KB1 + 8]
        qb1a_v = fblob[:, F_QB1A : F_QB1A + 1]
        qb1b_v = fblob[0:32, F_QB1B : F_QB1B + 1]
        qb2_v = fblob[0:MEL, F_QB2 : F_QB2 + 1]
        qb3s_v = fblob[0:MEL, F_QB3S : F_QB3S + 1]
        kb2_v = fblob[0:ATT, F_KB2 : F_KB2 + 1]

        keys1_bf = const.tile([128, 8, T2], bf16, tag="keys1")
        Qp = const.tile([MEL, T1], bf16, tag="Qp")
        kenc_sb = const.tile([ATT, T2], bf16, tag="kenc")
        sk_sb = const.tile([ATT, T2], bf16, tag="sk")
        k2neg_sb = const.tile([1, T2], bf16, tag="k2neg")
        q1a_bf = const.tile([128, T1], bf16, tag="q1a")
        q1b_bf = const.tile([32, T1], bf16, tag="q1b")
        q2_bf = const.tile([MEL, T1], bf16, tag="q2bf")

        # ---- warmup: lift the PE HAM clock gate while input DMAs land
        with tc.tile_pool(name="psW" + salt, bufs=1, space="PSUM") as psW:
            warm_ps = psW.tile([8, 512], f32, tag="warm")
            for _ in range(7):
                nc.tensor.matmul(
                    warm_ps, lhsT=warm_w, rhs=warm_rhs, start=True, stop=True
                )

        # ---- conv chains: query blocks serialise on their relus, so
        # conv1-over-keys co-blocks fill the gaps to keep the PE dense.
        with tc.tile_pool(name="psC" + salt, bufs=2, space="PSUM") as psC:
            k_ps = psC.tile([ATT, T2], f32, tag="kps", bufs=1)

            def q1_block(t0, psB):
                q1a_ps = psB.tile([128, 500], f32, tag="q1a", bufs=2)
                q1b_ps = psB.tile([32, 500], f32, tag="q1b", bufs=1)
                for k in range(3):
                    nc.tensor.matmul(
                        q1a_ps,
                        lhsT=wblob[0:MEL, W_QW1 + k * 160 : W_QW1 + k * 160 + 128],
                        rhs=qbuf[:, t0 + k : t0 + k + 500],
                        start=(k == 0),
                        stop=(k == 2),
                    )
                for k in range(3):
                    nc.tensor.matmul(
                        q1b_ps,
                        lhsT=wblob[0:MEL, W_QW1 + k * 160 + 128 : W_QW1 + k * 160 + 160],
                        rhs=qbuf[:, t0 + k : t0 + k + 500],
                        start=(k == 0),
                        stop=(k == 2),
                    )
                nc.vector.tensor_scalar(
                    q1a_bf[:, t0 : t0 + 500], q1a_ps, qb1a_v, 0.0,
                    op0=OP.add, op1=OP.max,
                )
                nc.scalar.activation(
                    q1b_bf[:, t0 : t0 + 500], q1b_ps, AF.Relu, bias=qb1b_v
                )

            def q2_block(t0, psB):
                q2_ps = psB.tile([MEL, 500], f32, tag="q2", bufs=1)
                nc.tensor.matmul(
                    q2_ps, lhsT=qw2a_v, rhs=q1a_bf[:, t0 : t0 + 500],
                    start=True, stop=False,
                )
                nc.tensor.matmul(
                    q2_ps, lhsT=qw2b_v, rhs=q1b_bf[:, t0 : t0 + 500],
                    start=False, stop=True,
                )
                nc.scalar.activation(
                    q2_bf[:, t0 : t0 + 500], q2_ps, AF.Relu, bias=qb2_v
                )

            def q3_block(t0, psB):
                q3_ps = psB.tile([MEL, 500], f32, tag="q3", bufs=1)
                nc.tensor.matmul(
                    q3_ps, lhsT=qw3_v, rhs=q2_bf[:, t0 : t0 + 500],
                    start=True, stop=True,
                )
                nc.scalar.activation(
                    Qp[:, t0 : t0 + 500], q3_ps, AF.Identity,
                    bias=qb3s_v, scale=2.0 * TEMP,
                )

            def conv1_block(co):
                c1_ps = psC.tile([128, T2], f32, tag="c1", bufs=2)
                n = 0
                for k in range(3):
                    for p in range(2):
                        nc.tensor.matmul(
                            c1_ps,
                            lhsT=kw1_sb[:, co, k, p],
                            rhs=keys_sb[:, 2 * p : 2 * p + 2, k : k + T2],
                            start=(n == 0),
                            stop=(n == 5),
                            perf_mode=DR,
                        )
                        n += 1
                if co % 2 == 0:
                    nc.scalar.activation(
                        keys1_bf[:, co, :], c1_ps, AF.Relu,
                        bias=kb1_v[:, co : co + 1],
                    )
                else:
                    nc.vector.tensor_scalar(
                        keys1_bf[:, co, :], c1_ps, kb1_v[:, co : co + 1], 0.0,
                        op0=OP.add, op1=OP.max,
                    )
                nc.tensor.matmul(
                    k_ps,
                    lhsT=kw2_v[:, co * ATT : (co + 1) * ATT],
                    rhs=keys1_bf[:, co, :],
                    start=(co == 0),
                    stop=(co == 7),
                )

            with tc.tile_pool(name="psB" + salt, bufs=1, space="PSUM") as psB:
                q1_block(0, psB)
                conv1_block(0)
                q1_block(500, psB)
                conv1_block(1)
                q2_block(0, psB)
                conv1_block(2)
                q2_block(500, psB)
                conv1_block(3)
                q3_block(0, psB)
                conv1_block(4)
                q3_block(500, psB)
            conv1_block(5)
            conv1_block(6)
            conv1_block(7)

            nc.scalar.activation(sk_sb, k_ps, AF.Square, bias=kb2_v)
            nc.vector.tensor_scalar_add(kenc_sb, k_ps, kb2_v)

        # ---- attention scores + double softmax epilogue, chunk-pipelined
        with (
            tc.tile_pool(name="psD" + salt, bufs=6, space="PSUM") as psD,
            tc.tile_pool(name="psK" + salt, bufs=1, space="PSUM") as psK,
        ):
            k2_ps = psK.tile([1, T2], f32, tag="k2")
            nc.tensor.matmul(k2_ps, lhsT=ones80, rhs=sk_sb, start=True, stop=True)
            nc.vector.tensor_scalar_mul(k2neg_sb, k2_ps, -TEMP)

            for c in range(8):
                rows = 128 if c < 7 else T1 - 7 * 128
                a_ps = psD.tile([128, T2], f32, tag="attn", bufs=6)
                nc.tensor.matmul(
                    a_ps[0:rows],
                    lhsT=Qp[:, c * 128 : c * 128 + rows],
                    rhs=kenc_sb,
                    start=True,
                    stop=False,
                )
                nc.tensor.matmul(
                    a_ps[0:rows],
                    lhsT=ones1[:, 0:rows],
                    rhs=k2neg_sb,
                    start=False,
                    stop=True,
                )
                e = work.tile([128, T2], bf16, tag="e", bufs=3)
                s = small.tile([128, 1], f32, tag="s")
                nc.scalar.activation(
                    e[0:rows], a_ps[0:rows], AF.Exp, accum_out=s[0:rows]
                )
                em = work.tile([128, T2], bf16, tag="em", bufs=3)
                s2 = small.tile([128, 1], f32, tag="s2")
                # em = e * pm, s2 = sum(em) in one DVE pass
                nc.vector.scalar_tensor_tensor(
                    out=em[0:rows],
                    in0=e[0:rows],
                    scalar=1.0,
                    in1=lppm[0:rows, 1, c, :],
                    op0=OP.mult,
                    op1=OP.mult,
                    accum_out=s2[0:rows],
                )
                lns = small.tile([128, 1], f32, tag="lns")
                nc.scalar.activation(lns[0:rows], s[0:rows], AF.Ln)
                ocomb = work.tile([128, 2, T2], bf16, tag="ocomb", bufs=3)
                # o_logp = (a - lnS) + log_prior, fused on DVE
                nc.vector.scalar_tensor_tensor(
                    out=ocomb[0:rows, 0, :],
                    in0=a_ps[0:rows],
                    scalar=lns[0:rows],
                    in1=lppm[0:rows, 0, c, :],
                    op0=OP.subtract,
                    op1=OP.add,
                )
                r2 = small.tile([128, 1], f32, tag="r2")
                nc.vector.reciprocal(r2[0:rows], s2[0:rows])
                nc.gpsimd.tensor_scalar_mul(
                    ocomb[0:rows, 1, :], em[0:rows], r2[0:rows]
                )
                nc.sync.dma_start(out=out_p[0:rows, c], in_=ocomb[0:rows])

    _split_excess_waits(nc)
    return nc


_NC_CACHE = {}


def _get_nc():
    if "nc" not in _NC_CACHE:
        _NC_CACHE["nc"] = _build_nc()
    return _NC_CACHE["nc"]


def _prep_in_maps(inputs):
    q = np.asarray(inputs["queries"], F32)
    k = np.asarray(inputs["keys"], F32)
    mask = np.asarray(inputs["mask"])
    prior = np.asarray(inputs["attn_prior"], F32)
    spk = np.asarray(inputs["speaker_embed"], F32)
    emo = np.asarray(inputs["emotion_embed"], F32)

    kw1 = np.asarray(inputs["kw1"], F32)
    kb1 = np.asarray(inputs["kb1"], F32)
    kw2 = np.asarray(inputs["kw2"], F32)
    kb2 = np.asarray(inputs["kb2"], F32)
    qw1 = np.asarray(inputs["qw1"], F32)
    qb1 = np.asarray(inputs["qb1"], F32)
    qw2 = np.asarray(inputs["qw2"], F32)
    qb2 = np.asarray(inputs["qb2"], F32)
    qw3 = np.asarray(inputs["qw3"], F32)
    qb3 = np.asarray(inputs["qb3"], F32)
    spk_kw = np.asarray(inputs["spk_kw"], F32)
    spk_kb = np.asarray(inputs["spk_kb"], F32)
    spk_qw = np.asarray(inputs["spk_qw"], F32)
    spk_qb = np.asarray(inputs["spk_qb"], F32)
    emo_kw = np.asarray(inputs["emo_kw"], F32)
    emo_kb = np.asarray(inputs["emo_kb"], F32)
    emo_qw = np.asarray(inputs["emo_qw"], F32)
    emo_qb = np.asarray(inputs["emo_qb"], F32)

    # conditioning folded into the activations (tiny per-batch GEMMs)
    kadd = spk @ spk_kw.T + spk_kb + emo @ emo_kw.T + emo_kb   # (B, 512)
    qadd = spk @ spk_qw.T + spk_qb + emo @ emo_qw.T + emo_qb   # (B, 80)
    keys_c = k + kadd[:, :, None]
    q_c = q + qadd[:, :, None]

    # conv windows pre-padded
    keys_pad = np.zeros((B, 4, 128, T2 + 2), F32)
    keys_pad[:, :, :, 1 : T2 + 1] = keys_c.reshape(B, 4, 128, T2)
    keys_t = np.clip(keys_pad.transpose(0, 2, 1, 3), -224.0, 224.0).astype(FP8)
    qpad = np.zeros((B, MEL, T1 + 2), F32)
    qpad[:, :, 1 : T1 + 1] = q_c
    qpad = qpad.astype(BF16)

    # conv1 weights: fp8 x16, DoubleRow pair layout [j, o, k, p, i, m]
    kw1_dr = np.ascontiguousarray(
        kw1.reshape(8, 128, 2, 2, 128, 3).transpose(4, 0, 5, 2, 3, 1) * WS
    ).astype(FP8)
    # conv2 weights carry the 1/WS un-scale
    kw2_t = np.ascontiguousarray(
        (kw2[:, :, 0].T / WS).reshape(8, 128, ATT).transpose(1, 0, 2)
    )  # [j, o, c]

    wblob = np.zeros((128, W_COLS), F32)
    wblob[:, W_KW2 : W_KW2 + 640] = kw2_t.reshape(128, 640)
    wblob[:, W_QW2A : W_QW2A + 80] = qw2[:, :, 0].T[0:128]
    wblob[0:MEL, W_QW1 : W_QW1 + 480] = qw1.transpose(1, 2, 0).reshape(MEL, 480)
    wblob[0:MEL, W_QW3 : W_QW3 + 80] = qw3[:, :, 0].T
    wblob[0:32, W_QW2B : W_QW2B + 80] = qw2[:, :, 0].T[128:160]
    wblob = wblob.astype(BF16)

    fblob = np.zeros((128, F_COLS), F32)
    fblob[:, F_KB1 : F_KB1 + 8] = (WS * kb1).reshape(8, 128).T
    fblob[:, F_QB1A] = qb1[0:128]
    fblob[0:32, F_QB1B] = qb1[128:160]
    fblob[0:MEL, F_QB2] = qb2
    fblob[0:MEL, F_QB3S] = 2.0 * TEMP * qb3
    fblob[0:ATT, F_KB2] = kb2

    # log-prior and masked prior, chunk-major [row-in-chunk, {lp,pm}, chunk, T2]
    lp = np.log(prior + 1e-8)                                   # (B, 1000, 256)
    pmm = (prior + 1e-8) * (1.0 - mask[:, :, 0].astype(F32))[:, None, :]
    lppm = np.zeros((B, 2, 1024, T2), F32)
    lppm[:, 0, :T1] = lp
    lppm[:, 1, :T1] = pmm
    # -> (B, 128, 2, 8, T2)
    lppm = np.ascontiguousarray(
        lppm.reshape(B, 2, 8, 128, T2).transpose(0, 3, 1, 2, 4)
    ).astype(BF16)

    shared = {
        "wblob": wblob,
        "fblob": np.ascontiguousarray(fblob),
        "kw1": kw1_dr,
    }

    in_maps = []
    for b in range(B):
        m = dict(shared)
        m["queries"] = np.ascontiguousarray(qpad[b])
        m["keys"] = np.ascontiguousarray(keys_t[b])
        m["lppm"] = np.ascontiguousarray(lppm[b])
        in_maps.append(m)
    return in_maps


def kernel(**inputs):
    from concourse.bass_utils import run_bass_kernel_spmd

    nc = _get_nc()
    in_maps = _prep_in_maps(inputs)
    res = run_bass_kernel_spmd(nc, in_maps, core_ids=list(range(N_CORES)))
    attn = np.empty((B, 1, T1, T2), F32)
    logp = np.empty((B, 1, T1, T2), F32)
    for i in range(N_CORES):
        o = np.asarray(res.results[i]["out"]).astype(F32)      # [128, 8, 2, T2]
        o = o.transpose(1, 0, 2, 3).reshape(1024, 2, T2)[:T1]  # [1000, 2, T2]
        logp[i, 0] = o[:, 0]
        attn[i, 0] = o[:, 1]
    return attn, logp


# revision 8
# speedup vs baseline: 1.4955x; 1.4955x over previous
"""AlignmentEncoder forward on 8 Trainium2 NeuronCores (data-parallel over batch).

Per batch b (one batch per core), with speaker/emotion conditioning folded
into keys/queries host-side:
  keys_enc = conv1d(relu(conv1d(keys, kw1, pad1)), kw2)                 (80, 256)
  queries_enc = conv1d(relu(conv1d(relu(conv1d(q,qw1,pad1)),qw2)),qw3)  (80, 1000)
  a[t,s] = 2*TEMP*qk - TEMP*k2[s]   (the q2 row-constant cancels in both
           log_softmax and softmax and is dropped)
  out_logp = a - ln(sum_s exp a) + log(prior + 1e-8)
  out_attn = e*pm / sum_s(e*pm),  e = exp(a), pm = (prior+1e-8)*(1-mask)

conv1-over-keys runs in fp8e4m3 with DoubleRow packing (weights scaled x16,
un-scaled through kb1*16 / kw2/16); everything else bf16 with f32 PSUM.
Outputs are written bf16 and widened host-side.  Input DMAs are split
across the SP and ACT HWDGE rings so dispatch serialisation doesn't gate
the conv start.
"""

import sys
from contextlib import ExitStack

sys.path.insert(0, "/opt/trn_rl_repo")

import numpy as np
import ml_dtypes

import bass_rust
from bass_rust import add_dep_helper
import concourse.bass as bass
import concourse.mybir as mybir
import concourse.tile as tile

BF16 = ml_dtypes.bfloat16
FP8 = ml_dtypes.float8_e4m3
F32 = np.float32

B, MEL, TXT, ATT, T1, T2 = 8, 80, 512, 80, 1000, 256
TEMP = 0.0005
N_CORES = 8
WS = 16.0  # fp8 weight scale for conv1

_MAX_WAITS = 1  # this walrus build rejects multi-wait instructions


def _split_excess_waits(nc):
    """Move excess sem waits from any instruction onto single-wait NoOps
    inserted immediately before it.  Mid-program instructions get
    same-engine NoOps (program order preserves wait semantics); the
    kernel-tail Drain's waits are spread across all engines — they run in
    parallel and the following all-engine barrier re-synchronises."""
    spread = [
        mybir.EngineType.SP,
        mybir.EngineType.Activation,
        mybir.EngineType.DVE,
        mybir.EngineType.PE,
        mybir.EngineType.Pool,
    ]
    uid = 0
    for blk in nc.m.functions[0].blocks:
        insts = list(blk.instructions)
        out = []
        changed = False
        for inst in insts:
            si = inst.sync_info
            waits = list(si.on_wait) if si is not None and si.on_wait else []
            if len(waits) > _MAX_WAITS:
                si.on_wait = waits[-_MAX_WAITS:]
                extra = waits[: -_MAX_WAITS]
                is_drain = isinstance(inst, mybir.InstDrain)
                for j in range(0, len(extra), _MAX_WAITS):
                    nop = mybir.InstNoOp(name=f"I-waitsplit-{uid}", ins=[], outs=[])
                    nop.engine = (
                        spread[(j // _MAX_WAITS) % len(spread)]
                        if is_drain
                        else inst.engine
                    )
                    uid += 1
                    nop.bass_nofuse = True
                    nop.sync_info = bass_rust.SyncInfo(
                        on_wait=extra[j : j + _MAX_WAITS], on_update=[]
                    )
                    out.append(nop)
                changed = True
            out.append(inst)
        if changed:
            blk.instructions = out


class _TC(tile.TileContext):
    pass


# wblob (bf16) column layout
W_KW2 = 0            # [128, 8*80]
W_QW2A = 640         # [128, 80]
W_QW1 = 720          # [80, 3*160]
W_QW3 = 1200         # [80, 80]
W_QW2B = 1280        # [32, 80]
W_COLS = 1360
# fblob (f32) column layout
F_KB1 = 0            # [128, 8]  (16*kb1)
F_QB1A = 8           # [128, 1]
F_QB1B = 9           # [32, 1]
F_QB2 = 10           # [80, 1]
F_QB3S = 11          # [80, 1]  (2*TEMP*qb3)
F_KB2 = 12           # [80, 1]
F_COLS = 13


def _build_nc(salt=""):
    f32 = mybir.dt.float32
    bf16 = mybir.dt.bfloat16
    fp8 = mybir.dt.float8e4
    AF = mybir.ActivationFunctionType
    OP = mybir.AluOpType
    DR = mybir.MatmulPerfMode.DoubleRow

    nc = bass.Bass("TRN2", target_bir_lowering=False, debug=False, num_devices=N_CORES)
    dp = nc.declare_dram_parameter
    queries_p = dp("queries", [MEL, T1 + 2], bf16, isOutput=False)
    keys_p = dp("keys", [128, 4, T2 + 2], fp8, isOutput=False)
    wblob_p = dp("wblob", [128, W_COLS], bf16, isOutput=False)
    fblob_p = dp("fblob", [128, F_COLS], f32, isOutput=False)
    kw1_p = dp("kw1", [128, 8, 3, 2, 2, 128], fp8, isOutput=False)
    lppm_p = dp("lppm", [128, 2, 8, T2], bf16, isOutput=False)
    out_p = dp("out", [128, 8, 2, T2], bf16, isOutput=True)

    with _TC(nc) as tc, ExitStack() as ctx:
        const = ctx.enter_context(tc.tile_pool(name="const" + salt, bufs=1))
        work = ctx.enter_context(tc.tile_pool(name="work" + salt, bufs=4))
        small = ctx.enter_context(tc.tile_pool(name="small" + salt, bufs=8))

        # ---- t=0 memsets: warm tiles on GpSimd (its stream dispatches
        # ~1us before Vector's), the rest on Vector.
        warm_w = const.tile([128, 8], bf16, tag="warmw")
        nc.gpsimd.memset(warm_w, 0.0)
        warm_rhs = const.tile([128, 512], bf16, tag="warmrhs")
        nc.gpsimd.memset(warm_rhs, 0.0)
        ones80 = const.tile([ATT, 1], bf16, tag="ones80")
        nc.vector.memset(ones80, 1.0)
        ones1 = const.tile([1, 128], bf16, tag="ones1")
        nc.vector.memset(ones1, 1.0)

        # ---- input DMAs.  SP ring: q-chain inputs + lppm; ACT ring: keys
        # + conv1 weights.  Dispatch order = need order; no completion
        # gating (the per-ring FIFO already serialises dispatch).
        qbuf = const.tile([MEL, T1 + 2], bf16, tag="qbuf")
        wblob = const.tile([128, W_COLS], bf16, tag="wblob")
        fblob = const.tile([128, F_COLS], f32, tag="fblob")
        keys_sb = const.tile([128, 4, T2 + 2], fp8, tag="keys")
        kw1_sb = const.tile([128, 8, 3, 2, 2, 128], fp8, tag="kw1")
        lppm = const.tile([128, 2, 8, T2], bf16, tag="lppm")
        nc.sync.dma_start(out=qbuf, in_=queries_p[:])
        nc.scalar.dma_start(out=keys_sb, in_=keys_p[:])
        nc.sync.dma_start(out=wblob, in_=wblob_p[:])
        for g in range(4):
            nc.scalar.dma_start(
                out=kw1_sb[:, 2 * g : 2 * g + 2], in_=kw1_p[:, 2 * g : 2 * g + 2]
            )
        nc.sync.dma_start(out=fblob, in_=fblob_p[:])
        nc.sync.dma_start(out=lppm, in_=lppm_p[:])

        # ACT-table preload: first scalar activation triggers the ~1.3us
        # table load; this runs right after the ACT-ring DMA dispatches,
        # well before the first relu needs it.
        dummy = const.tile([128, 2], f32, tag="dummy")
        nc.vector.memset(dummy, 1.0)
        nc.scalar.activation(dummy[:, 1:2], dummy[:, 0:1], AF.Exp)

        # weight views into the blobs
        kw2_v = wblob[:, W_KW2 : W_KW2 + 640]
        qw2a_v = wblob[:, W_QW2A : W_QW2A + 80]
        qw3_v = wblob[0:MEL, W_QW3 : W_QW3 + 80]
        qw2b_v = wblob[0:32, W_QW2B : W_QW2B + 80]
        kb1_v = fblob[:, F_KB1 : F_KB1 + 8]
        qb1a_v = fblob[:, F_QB1A : F_QB1A + 1]
        qb1b_v = fblob[0:32, F_QB1B : F_QB1B + 1]
        qb2_v = fblob[0:MEL, F_QB2 : F_QB2 + 1]
        qb3s_v = fblob[0:MEL, F_QB3S : F_QB3S + 1]
        kb2_v = fblob[0:ATT, F_KB2 : F_KB2 + 1]

        keys1_bf = const.tile([128, 8, T2], bf16, tag="keys1")
        Qp = const.tile([MEL, T1], bf16, tag="Qp")
        kenc_sb = const.tile([ATT, T2], bf16, tag="kenc")
        sk_sb = const.tile([ATT, T2], bf16, tag="sk")
        k2neg_sb = const.tile([1, T2], bf16, tag="k2neg")
        q1a_bf = const.tile([128, T1], bf16, tag="q1a")
        q1b_bf = const.tile([32, T1], bf16, tag="q1b")
        q2_bf = const.tile([MEL, T1], bf16, tag="q2bf")

        # ---- warmup: lift the PE HAM clock gate while input DMAs land
        with tc.tile_pool(name="psW" + salt, bufs=1, space="PSUM") as psW:
            warm_ps = psW.tile([8, 512], f32, tag="warm")
            for _ in range(7):
                nc.tensor.matmul(
                    warm_ps, lhsT=warm_w, rhs=warm_rhs, start=True, stop=True
                )

        # ---- conv chains: query blocks serialise on their relus, so
        # conv1-over-keys co-blocks fill the gaps to keep the PE dense.
        with tc.tile_pool(name="psC" + salt, bufs=2, space="PSUM") as psC:
            k_ps = psC.tile([ATT, T2], f32, tag="kps", bufs=1)

            def q1_block(t0, psB):
                q1a_ps = psB.tile([128, 500], f32, tag="q1a", bufs=2)
                q1b_ps = psB.tile([32, 500], f32, tag="q1b", bufs=1)
                for k in range(3):
                    nc.tensor.matmul(
                        q1a_ps,
                        lhsT=wblob[0:MEL, W_QW1 + k * 160 : W_QW1 + k * 160 + 128],
                        rhs=qbuf[:, t0 + k : t0 + k + 500],
                        start=(k == 0),
                        stop=(k == 2),
                    )
                for k in range(3):
                    nc.tensor.matmul(
                        q1b_ps,
                        lhsT=wblob[0:MEL, W_QW1 + k * 160 + 128 : W_QW1 + k * 160 + 160],
                        rhs=qbuf[:, t0 + k : t0 + k + 500],
                        start=(k == 0),
                        stop=(k == 2),
                    )
                nc.vector.tensor_scalar(
                    q1a_bf[:, t0 : t0 + 500], q1a_ps, qb1a_v, 0.0,
                    op0=OP.add, op1=OP.max,
                )
                nc.scalar.activation(
                    q1b_bf[:, t0 : t0 + 500], q1b_ps, AF.Relu, bias=qb1b_v
                )

            def q2_block(t0, psB):
                q2_ps = psB.tile([MEL, 500], f32, tag="q2", bufs=1)
                nc.tensor.matmul(
                    q2_ps, lhsT=qw2a_v, rhs=q1a_bf[:, t0 : t0 + 500],
                    start=True, stop=False,
                )
                nc.tensor.matmul(
                    q2_ps, lhsT=qw2b_v, rhs=q1b_bf[:, t0 : t0 + 500],
                    start=False, stop=True,
                )
                nc.scalar.activation(
                    q2_bf[:, t0 : t0 + 500], q2_ps, AF.Relu, bias=qb2_v
                )

            def q3_block(t0, psB):
                q3_ps = psB.tile([MEL, 500], f32, tag="q3", bufs=1)
                nc.tensor.matmul(
                    q3_ps, lhsT=qw3_v, rhs=q2_bf[:, t0 : t0 + 500],
                    start=True, stop=True,
                )
                nc.scalar.activation(
                    Qp[:, t0 : t0 + 500], q3_ps, AF.Identity,
                    bias=qb3s_v, scale=2.0 * TEMP,
                )

            def conv1_block(co):
                c1_ps = psC.tile([128, T2], f32, tag="c1", bufs=2)
                n = 0
                for k in range(3):
                    for p in range(2):
                        nc.tensor.matmul(
                            c1_ps,
                            lhsT=kw1_sb[:, co, k, p],
                            rhs=keys_sb[:, 2 * p : 2 * p + 2, k : k + T2],
                            start=(n == 0),
                            stop=(n == 5),
                            perf_mode=DR,
                        )
                        n += 1
                if co % 2 == 0:
                    nc.scalar.activation(
                        keys1_bf[:, co, :], c1_ps, AF.Relu,
                        bias=kb1_v[:, co : co + 1],
                    )
                else:
                    nc.vector.tensor_scalar(
                        keys1_bf[:, co, :], c1_ps, kb1_v[:, co : co + 1], 0.0,
                        op0=OP.add, op1=OP.max,
                    )
                nc.tensor.matmul(
                    k_ps,
                    lhsT=kw2_v[:, co * ATT : (co + 1) * ATT],
                    rhs=keys1_bf[:, co, :],
                    start=(co == 0),
                    stop=(co == 7),
                )

            with tc.tile_pool(name="psB" + salt, bufs=1, space="PSUM") as psB:
                q1_block(0, psB)
                conv1_block(0)
                q1_block(500, psB)
                conv1_block(1)
                q2_block(0, psB)
                conv1_block(2)
                q2_block(500, psB)
                conv1_block(3)
                q3_block(0, psB)
                conv1_block(4)
                q3_block(500, psB)
            conv1_block(5)
            conv1_block(6)
            conv1_block(7)

            nc.scalar.activation(sk_sb, k_ps, AF.Square, bias=kb2_v)
            nc.vector.tensor_scalar_add(kenc_sb, k_ps, kb2_v)

        # ---- attention scores + double softmax epilogue, chunk-pipelined
        with (
            tc.tile_pool(name="psD" + salt, bufs=6, space="PSUM") as psD,
            tc.tile_pool(name="psK" + salt, bufs=1, space="PSUM") as psK,
        ):
            k2_ps = psK.tile([1, T2], f32, tag="k2")
            nc.tensor.matmul(k2_ps, lhsT=ones80, rhs=sk_sb, start=True, stop=True)
            nc.vector.tensor_scalar_mul(k2neg_sb, k2_ps, -TEMP)

            for c in range(8):
                rows = 128 if c < 7 else T1 - 7 * 128
                a_ps = psD.tile([128, T2], f32, tag="attn", bufs=6)
                nc.tensor.matmul(
                    a_ps[0:rows],
                    lhsT=Qp[:, c * 128 : c * 128 + rows],
                    rhs=kenc_sb,
                    start=True,
                    stop=False,
                )
                nc.tensor.matmul(
                    a_ps[0:rows],
                    lhsT=ones1[:, 0:rows],
                    rhs=k2neg_sb,
                    start=False,
                    stop=True,
                )
                e = work.tile([128, T2], bf16, tag="e", bufs=3)
                s = small.tile([128, 1], f32, tag="s")
                nc.scalar.activation(
                    e[0:rows], a_ps[0:rows], AF.Exp, accum_out=s[0:rows]
                )
                em = work.tile([128, T2], bf16, tag="em", bufs=3)
                s2 = small.tile([128, 1], f32, tag="s2")
                # em = e * pm, s2 = sum(em) in one DVE pass
                nc.vector.scalar_tensor_tensor(
                    out=em[0:rows],
                    in0=e[0:rows],
                    scalar=1.0,
                    in1=lppm[0:rows, 1, c, :],
                    op0=OP.mult,
                    op1=OP.mult,
                    accum_out=s2[0:rows],
                )
                lns = small.tile([128, 1], f32, tag="lns")
                nc.scalar.activation(lns[0:rows], s[0:rows], AF.Ln)
                ocomb = work.tile([128, 2, T2], bf16, tag="ocomb", bufs=3)
                # o_logp = (a - lnS) + log_prior, fused on DVE
                nc.vector.scalar_tensor_tensor(
                    out=ocomb[0:rows, 0, :],
                    in0=a_ps[0:rows],
                    scalar=lns[0:rows],
                    in1=lppm[0:rows, 0, c, :],
                    op0=OP.subtract,
                    op1=OP.add,
                )
                r2 = small.tile([128, 1], f32, tag="r2")
                nc.vector.reciprocal(r2[0:rows], s2[0:rows])
                nc.vector.tensor_scalar_mul(
                    ocomb[0:rows, 1, :], em[0:rows], r2[0:rows]
                )
                nc.sync.dma_start(out=out_p[0:rows, c], in_=ocomb[0:rows])

    _split_excess_waits(nc)
    return nc


_NC_CACHE = {}


def _get_nc():
    if "nc" not in _NC_CACHE:
        _NC_CACHE["nc"] = _build_nc()
    return _NC_CACHE["nc"]


def _prep_in_maps(inputs):
    q = np.asarray(inputs["queries"], F32)
    k = np.asarray(inputs["keys"], F32)
    mask = np.asarray(inputs["mask"])
    prior = np.asarray(inputs["attn_prior"], F32)
    spk = np.asarray(inputs["speaker_embed"], F32)
    emo = np.asarray(inputs["emotion_embed"], F32)

    kw1 = np.asarray(inputs["kw1"], F32)
    kb1 = np.asarray(inputs["kb1"], F32)
    kw2 = np.asarray(inputs["kw2"], F32)
    kb2 = np.asarray(inputs["kb2"], F32)
    qw1 = np.asarray(inputs["qw1"], F32)
    qb1 = np.asarray(inputs["qb1"], F32)
    qw2 = np.asarray(inputs["qw2"], F32)
    qb2 = np.asarray(inputs["qb2"], F32)
    qw3 = np.asarray(inputs["qw3"], F32)
    qb3 = np.asarray(inputs["qb3"], F32)
    spk_kw = np.asarray(inputs["spk_kw"], F32)
    spk_kb = np.asarray(inputs["spk_kb"], F32)
    spk_qw = np.asarray(inputs["spk_qw"], F32)
    spk_qb = np.asarray(inputs["spk_qb"], F32)
    emo_kw = np.asarray(inputs["emo_kw"], F32)
    emo_kb = np.asarray(inputs["emo_kb"], F32)
    emo_qw = np.asarray(inputs["emo_qw"], F32)
    emo_qb = np.asarray(inputs["emo_qb"], F32)

    # conditioning folded into the activations (tiny per-batch GEMMs)
    kadd = spk @ spk_kw.T + spk_kb + emo @ emo_kw.T + emo_kb   # (B, 512)
    qadd = spk @ spk_qw.T + spk_qb + emo @ emo_qw.T + emo_qb   # (B, 80)
    keys_c = k + kadd[:, :, None]
    q_c = q + qadd[:, :, None]

    # conv windows pre-padded
    keys_pad = np.zeros((B, 4, 128, T2 + 2), F32)
    keys_pad[:, :, :, 1 : T2 + 1] = keys_c.reshape(B, 4, 128, T2)
    keys_t = np.clip(keys_pad.transpose(0, 2, 1, 3), -224.0, 224.0).astype(FP8)
    qpad = np.zeros((B, MEL, T1 + 2), F32)
    qpad[:, :, 1 : T1 + 1] = q_c
    qpad = qpad.astype(BF16)

    # conv1 weights: fp8 x16, DoubleRow pair layout [j, o, k, p, i, m]
    kw1_dr = np.ascontiguousarray(
        kw1.reshape(8, 128, 2, 2, 128, 3).transpose(4, 0, 5, 2, 3, 1) * WS
    ).astype(FP8)
    # conv2 weights carry the 1/WS un-scale
    kw2_t = np.ascontiguousarray(
        (kw2[:, :, 0].T / WS).reshape(8, 128, ATT).transpose(1, 0, 2)
    )  # [j, o, c]

    wblob = np.zeros((128, W_COLS), F32)
    wblob[:, W_KW2 : W_KW2 + 640] = kw2_t.reshape(128, 640)
    wblob[:, W_QW2A : W_QW2A + 80] = qw2[:, :, 0].T[0:128]
    wblob[0:MEL, W_QW1 : W_QW1 + 480] = qw1.transpose(1, 2, 0).reshape(MEL, 480)
    wblob[0:MEL, W_QW3 : W_QW3 + 80] = qw3[:, :, 0].T
    wblob[0:32, W_QW2B : W_QW2B + 80] = qw2[:, :, 0].T[128:160]
    wblob = wblob.astype(BF16)

    fblob = np.zeros((128, F_COLS), F32)
    fblob[:, F_KB1 : F_KB1 + 8] = (WS * kb1).reshape(8, 128).T
    fblob[:, F_QB1A] = qb1[0:128]
    fblob[0:32, F_QB1B] = qb1[128:160]
    fblob[0:MEL, F_QB2] = qb2
    fblob[0:MEL, F_QB3S] = 2.0 * TEMP * qb3
    fblob[0:ATT, F_KB2] = kb2

    # log-prior and masked prior, chunk-major [row-in-chunk, {lp,pm}, chunk, T2]
    lp = np.log(prior + 1e-8)                                   # (B, 1000, 256)
    pmm = (prior + 1e-8) * (1.0 - mask[:, :, 0].astype(F32))[:, None, :]
    lppm = np.zeros((B, 2, 1024, T2), F32)
    lppm[:, 0, :T1] = lp
    lppm[:, 1, :T1] = pmm
    # -> (B, 128, 2, 8, T2)
    lppm = np.ascontiguousarray(
        lppm.reshape(B, 2, 8, 128, T2).transpose(0, 3, 1, 2, 4)
    ).astype(BF16)

    shared = {
        "wblob": wblob,
        "fblob": np.ascontiguousarray(fblob),
        "kw1": kw1_dr,
    }

    in_maps = []
    for b in range(B):
        m = dict(shared)
        m["queries"] = np.ascontiguousarray(qpad[b])
        m["keys"] = np.ascontiguousarray(keys_t[b])
        m["lppm"] = np.ascontiguousarray(lppm[b])
        in_maps.append(m)
    return in_maps


def kernel(**inputs):
    from concourse.bass_utils import run_bass_kernel_spmd

    nc = _get_nc()
    in_maps = _prep_in_maps(inputs)
    res = run_bass_kernel_spmd(nc, in_maps, core_ids=list(range(N_CORES)))
    attn = np.empty((B, 1, T1, T2), F32)
    logp = np.empty((B, 1, T1, T2), F32)
    for i in range(N_CORES):
        o = np.asarray(res.results[i]["out"]).astype(F32)      # [128, 8, 2, T2]
        o = o.transpose(1, 0, 2, 3).reshape(1024, 2, T2)[:T1]  # [1000, 2, T2]
        logp[i, 0] = o[:, 0]
        attn[i, 0] = o[:, 1]
    return attn, logp


# revision 14
# speedup vs baseline: 1.6739x; 1.1192x over previous
"""AlignmentEncoder forward on 8 Trainium2 NeuronCores (data-parallel over batch).

Per batch b (one batch per core), with speaker/emotion conditioning folded
into keys/queries host-side:
  keys_enc = conv1d(relu(conv1d(keys, kw1, pad1)), kw2)                 (80, 256)
  queries_enc = conv1d(relu(conv1d(relu(conv1d(q,qw1,pad1)),qw2)),qw3)  (80, 1000)
  a[t,s] = 2*TEMP*qk - TEMP*k2[s]   (the q2 row-constant cancels in both
           log_softmax and softmax and is dropped)
  out_logp = a - ln(sum_s exp a) + log(prior + 1e-8)
  out_attn = e*pm / sum_s(e*pm),  e = exp(a), pm = (prior+1e-8)*(1-mask)

conv1-over-keys runs in fp8e4m3 with DoubleRow packing (weights scaled x16,
un-scaled through kb1*16 / kw2/16); everything else bf16 with f32 PSUM.
Outputs are written bf16 and widened host-side.  Input DMAs are split
across the SP and ACT HWDGE rings so dispatch serialisation doesn't gate
the conv start.
"""

import sys
from contextlib import ExitStack

sys.path.insert(0, "/opt/trn_rl_repo")

import numpy as np
import ml_dtypes

import bass_rust
from bass_rust import add_dep_helper
import concourse.bass as bass
import concourse.mybir as mybir
import concourse.tile as tile

BF16 = ml_dtypes.bfloat16
FP8 = ml_dtypes.float8_e4m3
F32 = np.float32

B, MEL, TXT, ATT, T1, T2 = 8, 80, 512, 80, 1000, 256
TEMP = 0.0005
N_CORES = 8
WS = 16.0  # fp8 weight scale for conv1

_MAX_WAITS = 1  # this walrus build rejects multi-wait instructions


def _split_excess_waits(nc):
    """Move excess sem waits from any instruction onto single-wait NoOps
    inserted immediately before it.  Mid-program instructions get
    same-engine NoOps (program order preserves wait semantics); the
    kernel-tail Drain's waits are spread across all engines — they run in
    parallel and the following all-engine barrier re-synchronises."""
    spread = [
        mybir.EngineType.SP,
        mybir.EngineType.Activation,
        mybir.EngineType.DVE,
        mybir.EngineType.PE,
        mybir.EngineType.Pool,
    ]
    uid = 0
    for blk in nc.m.functions[0].blocks:
        insts = list(blk.instructions)
        out = []
        changed = False
        for inst in insts:
            si = inst.sync_info
            waits = list(si.on_wait) if si is not None and si.on_wait else []
            if len(waits) > _MAX_WAITS:
                si.on_wait = waits[-_MAX_WAITS:]
                extra = waits[: -_MAX_WAITS]
                is_drain = isinstance(inst, mybir.InstDrain)
                for j in range(0, len(extra), _MAX_WAITS):
                    nop = mybir.InstNoOp(name=f"I-waitsplit-{uid}", ins=[], outs=[])
                    nop.engine = (
                        spread[(j // _MAX_WAITS) % len(spread)]
                        if is_drain
                        else inst.engine
                    )
                    uid += 1
                    nop.bass_nofuse = True
                    nop.sync_info = bass_rust.SyncInfo(
                        on_wait=extra[j : j + _MAX_WAITS], on_update=[]
                    )
                    out.append(nop)
                changed = True
            out.append(inst)
        if changed:
            blk.instructions = out


class _TC(tile.TileContext):
    pass


# wblob (bf16) column layout
W_KW2 = 0            # [128, 8*80]
W_QW2A = 640         # [128, 80]
W_QW1 = 720          # [80, 3*160]
W_QW3 = 1200         # [80, 80]
W_QW2B = 1280        # [32, 80]
W_COLS = 1360
# fblob (f32) column layout
F_KB1 = 0            # [128, 8]  (16*kb1)
F_QB1A = 8           # [128, 1]
F_QB1B = 9           # [32, 1]
F_QB2 = 10           # [80, 1]
F_QB3S = 11          # [80, 1]  (2*TEMP*qb3)
F_KB2 = 12           # [80, 1]
F_COLS = 13


def _build_nc(salt=""):
    f32 = mybir.dt.float32
    bf16 = mybir.dt.bfloat16
    fp8 = mybir.dt.float8e4
    AF = mybir.ActivationFunctionType
    OP = mybir.AluOpType
    DR = mybir.MatmulPerfMode.DoubleRow

    nc = bass.Bass("TRN2", target_bir_lowering=False, debug=False, num_devices=N_CORES)
    dp = nc.declare_dram_parameter
    queries_p = dp("queries", [MEL, T1 + 2], bf16, isOutput=False)
    keys_p = dp("keys", [128, 4, T2 + 2], fp8, isOutput=False)
    wblob_p = dp("wblob", [128, W_COLS], bf16, isOutput=False)
    fblob_p = dp("fblob", [128, F_COLS], f32, isOutput=False)
    kw1_p = dp("kw1", [128, 8, 3, 2, 2, 128], fp8, isOutput=False)
    lppm_p = dp("lppm", [128, 2, 8, T2], bf16, isOutput=False)
    out_p = dp("out", [128, 8, 2, T2], bf16, isOutput=True)

    with _TC(nc) as tc, ExitStack() as ctx:
        const = ctx.enter_context(tc.tile_pool(name="const" + salt, bufs=1))
        work = ctx.enter_context(tc.tile_pool(name="work" + salt, bufs=4))
        small = ctx.enter_context(tc.tile_pool(name="small" + salt, bufs=8))

        # ---- t=0 memsets: warm tiles on GpSimd (its stream dispatches
        # ~1us before Vector's), the rest on Vector.
        warm_w = const.tile([128, 8], bf16, tag="warmw")
        nc.gpsimd.memset(warm_w, 0.0)
        warm_rhs = const.tile([128, 512], bf16, tag="warmrhs")
        nc.gpsimd.memset(warm_rhs, 0.0)
        ones80 = const.tile([ATT, 1], bf16, tag="ones80")
        nc.vector.memset(ones80, 1.0)
        ones1 = const.tile([1, 128], bf16, tag="ones1")
        nc.vector.memset(ones1, 1.0)

        # ACT-table preload: first scalar activation triggers the ~1.3us
        # table load; issuing it at t=0 pulls it off the critical path.
        dummy = const.tile([128, 2], f32, tag="dummy")
        nc.vector.memset(dummy, 1.0)
        nc.scalar.activation(dummy[:, 1:2], dummy[:, 0:1], AF.Exp)

        # ---- input DMAs on the SP ring, dispatch order = need order.
        # kw1 is not completion-gated (ring FIFO already orders dispatch);
        # only the late-needed lppm is held back so its 1MB transfer can't
        # steal bandwidth from the conv-critical loads.
        qbuf = const.tile([MEL, T1 + 2], bf16, tag="qbuf")
        wblob = const.tile([128, W_COLS], bf16, tag="wblob")
        fblob = const.tile([128, F_COLS], f32, tag="fblob")
        keys_sb = const.tile([128, 4, T2 + 2], fp8, tag="keys")
        kw1_sb = const.tile([128, 8, 3, 2, 2, 128], fp8, tag="kw1")
        lppm = const.tile([128, 2, 8, T2], bf16, tag="lppm")
        nc.sync.dma_start(out=qbuf, in_=queries_p[:])
        nc.sync.dma_start(out=wblob, in_=wblob_p[:])
        nc.sync.dma_start(out=fblob, in_=fblob_p[:])
        nc.sync.dma_start(out=keys_sb, in_=keys_p[:])
        kw1_dmas = []
        for g in range(4):
            d = nc.sync.dma_start(
                out=kw1_sb[:, 2 * g : 2 * g + 2], in_=kw1_p[:, 2 * g : 2 * g + 2]
            )
            kw1_dmas.append(d)
        d = nc.sync.dma_start(out=lppm, in_=lppm_p[:])
        add_dep_helper(d.ins, kw1_dmas[3].ins, reason="dma staging: lppm after kw1")

        # weight views into the blobs
        kw2_v = wblob[:, W_KW2 : W_KW2 + 640]
        qw2a_v = wblob[:, W_QW2A : W_QW2A + 80]
        qw3_v = wblob[0:MEL, W_QW3 : W_QW3 + 80]
        qw2b_v = wblob[0:32, W_QW2B : W_QW2B + 80]
        kb1_v = fblob[:, F_KB1 : F_KB1 + 8]
        qb1a_v = fblob[:, F_QB1A : F_QB1A + 1]
        qb1b_v = fblob[0:32, F_QB1B : F_QB1B + 1]
        qb2_v = fblob[0:MEL, F_QB2 : F_QB2 + 1]
        qb3s_v = fblob[0:MEL, F_QB3S : F_QB3S + 1]
        kb2_v = fblob[0:ATT, F_KB2 : F_KB2 + 1]

        keys1_bf = const.tile([128, 8, T2], bf16, tag="keys1")
        Qp = const.tile([MEL, T1], bf16, tag="Qp")
        kenc_sb = const.tile([ATT, T2], bf16, tag="kenc")
        sk_sb = const.tile([ATT, T2], bf16, tag="sk")
        k2neg_sb = const.tile([1, T2], bf16, tag="k2neg")
        q1a_bf = const.tile([128, T1], bf16, tag="q1a")
        q1b_bf = const.tile([32, T1], bf16, tag="q1b")
        q2_bf = const.tile([MEL, T1], bf16, tag="q2bf")

        # ---- warmup: lift the PE HAM clock gate while input DMAs land
        with tc.tile_pool(name="psW" + salt, bufs=1, space="PSUM") as psW:
            warm_ps = psW.tile([8, 512], f32, tag="warm")
            for _ in range(9):
                nc.tensor.matmul(
                    warm_ps, lhsT=warm_w, rhs=warm_rhs, start=True, stop=True
                )

        # ---- conv chains: query blocks serialise on their relus, so
        # conv1-over-keys co-blocks fill the gaps to keep the PE dense.
        with (
            tc.tile_pool(name="psB" + salt, bufs=1, space="PSUM") as psB,
            tc.tile_pool(name="psC" + salt, bufs=2, space="PSUM") as psC,
        ):
            k_ps = psC.tile([ATT, T2], f32, tag="kps", bufs=1)

            def q1_block(t0, psB=psB):
                q1a_ps = psB.tile([128, 500], f32, tag="q1a", bufs=2)
                q1b_ps = psB.tile([32, 500], f32, tag="q1b", bufs=1)
                for k in range(3):
                    nc.tensor.matmul(
                        q1a_ps,
                        lhsT=wblob[0:MEL, W_QW1 + k * 160 : W_QW1 + k * 160 + 128],
                        rhs=qbuf[:, t0 + k : t0 + k + 500],
                        start=(k == 0),
                        stop=(k == 2),
                    )
                for k in range(3):
                    nc.tensor.matmul(
                        q1b_ps,
                        lhsT=wblob[0:MEL, W_QW1 + k * 160 + 128 : W_QW1 + k * 160 + 160],
                        rhs=qbuf[:, t0 + k : t0 + k + 500],
                        start=(k == 0),
                        stop=(k == 2),
                    )
                nc.vector.tensor_scalar(
                    q1a_bf[:, t0 : t0 + 500], q1a_ps, qb1a_v, 0.0,
                    op0=OP.add, op1=OP.max,
                )
                nc.scalar.activation(
                    q1b_bf[:, t0 : t0 + 500], q1b_ps, AF.Relu, bias=qb1b_v
                )

            def q2_block(t0, psB=psB):
                q2_ps = psB.tile([MEL, 500], f32, tag="q2", bufs=1)
                nc.tensor.matmul(
                    q2_ps, lhsT=qw2a_v, rhs=q1a_bf[:, t0 : t0 + 500],
                    start=True, stop=False,
                )
                nc.tensor.matmul(
                    q2_ps, lhsT=qw2b_v, rhs=q1b_bf[:, t0 : t0 + 500],
                    start=False, stop=True,
                )
                nc.scalar.activation(
                    q2_bf[:, t0 : t0 + 500], q2_ps, AF.Relu, bias=qb2_v
                )

            def q3_block(t0, psB=psB):
                q3_ps = psB.tile([MEL, 500], f32, tag="q3", bufs=1)
                nc.tensor.matmul(
                    q3_ps, lhsT=qw3_v, rhs=q2_bf[:, t0 : t0 + 500],
                    start=True, stop=True,
                )
                nc.scalar.activation(
                    Qp[:, t0 : t0 + 500], q3_ps, AF.Identity,
                    bias=qb3s_v, scale=2.0 * TEMP,
                )

            def conv1_block(co):
                c1_ps = psC.tile([128, T2], f32, tag="c1", bufs=2)
                n = 0
                for k in range(3):
                    for p in range(2):
                        nc.tensor.matmul(
                            c1_ps,
                            lhsT=kw1_sb[:, co, k, p],
                            rhs=keys_sb[:, 2 * p : 2 * p + 2, k : k + T2],
                            start=(n == 0),
                            stop=(n == 5),
                            perf_mode=DR,
                        )
                        n += 1
                if co % 2 == 0:
                    nc.scalar.activation(
                        keys1_bf[:, co, :], c1_ps, AF.Relu,
                        bias=kb1_v[:, co : co + 1],
                    )
                else:
                    nc.vector.tensor_scalar(
                        keys1_bf[:, co, :], c1_ps, kb1_v[:, co : co + 1], 0.0,
                        op0=OP.add, op1=OP.max,
                    )
                nc.tensor.matmul(
                    k_ps,
                    lhsT=kw2_v[:, co * ATT : (co + 1) * ATT],
                    rhs=keys1_bf[:, co, :],
                    start=(co == 0),
                    stop=(co == 7),
                )

            q1_block(0)
            conv1_block(0)
            q1_block(500)
            conv1_block(1)
            q2_block(0)
            conv1_block(2)
            q2_block(500)
            conv1_block(3)
            q3_block(0)
            conv1_block(4)
            q3_block(500)
            conv1_block(5)
            conv1_block(6)
            conv1_block(7)

            nc.scalar.activation(sk_sb, k_ps, AF.Square, bias=kb2_v)
            nc.vector.tensor_scalar_add(kenc_sb, k_ps, kb2_v)

        # ---- attention scores + double softmax epilogue, chunk-pipelined
        with (
            tc.tile_pool(name="psD" + salt, bufs=6, space="PSUM") as psD,
            tc.tile_pool(name="psK" + salt, bufs=1, space="PSUM") as psK,
        ):
            k2_ps = psK.tile([1, T2], f32, tag="k2")
            nc.tensor.matmul(k2_ps, lhsT=ones80, rhs=sk_sb, start=True, stop=True)
            nc.vector.tensor_scalar_mul(k2neg_sb, k2_ps, -TEMP)

            for c in range(8):
                rows = 128 if c < 7 else T1 - 7 * 128
                a_ps = psD.tile([128, T2], f32, tag="attn", bufs=6)
                nc.tensor.matmul(
                    a_ps[0:rows],
                    lhsT=Qp[:, c * 128 : c * 128 + rows],
                    rhs=kenc_sb,
                    start=True,
                    stop=False,
                )
                nc.tensor.matmul(
                    a_ps[0:rows],
                    lhsT=ones1[:, 0:rows],
                    rhs=k2neg_sb,
                    start=False,
                    stop=True,
                )
                e = work.tile([128, T2], bf16, tag="e", bufs=3)
                s = small.tile([128, 1], f32, tag="s")
                nc.scalar.activation(
                    e[0:rows], a_ps[0:rows], AF.Exp, accum_out=s[0:rows]
                )
                em = work.tile([128, T2], bf16, tag="em", bufs=3)
                s2 = small.tile([128, 1], f32, tag="s2")
                # em = e * pm, s2 = sum(em) in one DVE pass
                nc.vector.scalar_tensor_tensor(
                    out=em[0:rows],
                    in0=e[0:rows],
                    scalar=1.0,
                    in1=lppm[0:rows, 1, c, :],
                    op0=OP.mult,
                    op1=OP.mult,
                    accum_out=s2[0:rows],
                )
                lns = small.tile([128, 1], f32, tag="lns")
                nc.scalar.activation(lns[0:rows], s[0:rows], AF.Ln)
                ocomb = work.tile([128, 2, T2], bf16, tag="ocomb", bufs=3)
                # o_logp = (a - lnS) + log_prior, fused on DVE
                nc.vector.scalar_tensor_tensor(
                    out=ocomb[0:rows, 0, :],
                    in0=a_ps[0:rows],
                    scalar=lns[0:rows],
                    in1=lppm[0:rows, 0, c, :],
                    op0=OP.subtract,
                    op1=OP.add,
                )
                r2 = small.tile([128, 1], f32, tag="r2")
                nc.vector.reciprocal(r2[0:rows], s2[0:rows])
                nc.vector.tensor_scalar_mul(
                    ocomb[0:rows, 1, :], em[0:rows], r2[0:rows]
                )
                nc.sync.dma_start(out=out_p[0:rows, c], in_=ocomb[0:rows])

    _split_excess_waits(nc)
    return nc


_NC_CACHE = {}


def _get_nc():
    if "nc" not in _NC_CACHE:
        _NC_CACHE["nc"] = _build_nc()
    return _NC_CACHE["nc"]


def _prep_in_maps(inputs):
    q = np.asarray(inputs["queries"], F32)
    k = np.asarray(inputs["keys"], F32)
    mask = np.asarray(inputs["mask"])
    prior = np.asarray(inputs["attn_prior"], F32)
    spk = np.asarray(inputs["speaker_embed"], F32)
    emo = np.asarray(inputs["emotion_embed"], F32)

    kw1 = np.asarray(inputs["kw1"], F32)
    kb1 = np.asarray(inputs["kb1"], F32)
    kw2 = np.asarray(inputs["kw2"], F32)
    kb2 = np.asarray(inputs["kb2"], F32)
    qw1 = np.asarray(inputs["qw1"], F32)
    qb1 = np.asarray(inputs["qb1"], F32)
    qw2 = np.asarray(inputs["qw2"], F32)
    qb2 = np.asarray(inputs["qb2"], F32)
    qw3 = np.asarray(inputs["qw3"], F32)
    qb3 = np.asarray(inputs["qb3"], F32)
    spk_kw = np.asarray(inputs["spk_kw"], F32)
    spk_kb = np.asarray(inputs["spk_kb"], F32)
    spk_qw = np.asarray(inputs["spk_qw"], F32)
    spk_qb = np.asarray(inputs["spk_qb"], F32)
    emo_kw = np.asarray(inputs["emo_kw"], F32)
    emo_kb = np.asarray(inputs["emo_kb"], F32)
    emo_qw = np.asarray(inputs["emo_qw"], F32)
    emo_qb = np.asarray(inputs["emo_qb"], F32)

    # conditioning folded into the activations (tiny per-batch GEMMs)
    kadd = spk @ spk_kw.T + spk_kb + emo @ emo_kw.T + emo_kb   # (B, 512)
    qadd = spk @ spk_qw.T + spk_qb + emo @ emo_qw.T + emo_qb   # (B, 80)
    keys_c = k + kadd[:, :, None]
    q_c = q + qadd[:, :, None]

    # conv windows pre-padded
    keys_pad = np.zeros((B, 4, 128, T2 + 2), F32)
    keys_pad[:, :, :, 1 : T2 + 1] = keys_c.reshape(B, 4, 128, T2)
    keys_t = np.clip(keys_pad.transpose(0, 2, 1, 3), -224.0, 224.0).astype(FP8)
    qpad = np.zeros((B, MEL, T1 + 2), F32)
    qpad[:, :, 1 : T1 + 1] = q_c
    qpad = qpad.astype(BF16)

    # conv1 weights: fp8 x16, DoubleRow pair layout [j, o, k, p, i, m]
    kw1_dr = np.ascontiguousarray(
        kw1.reshape(8, 128, 2, 2, 128, 3).transpose(4, 0, 5, 2, 3, 1) * WS
    ).astype(FP8)
    # conv2 weights carry the 1/WS un-scale
    kw2_t = np.ascontiguousarray(
        (kw2[:, :, 0].T / WS).reshape(8, 128, ATT).transpose(1, 0, 2)
    )  # [j, o, c]

    wblob = np.zeros((128, W_COLS), F32)
    wblob[:, W_KW2 : W_KW2 + 640] = kw2_t.reshape(128, 640)
    wblob[:, W_QW2A : W_QW2A + 80] = qw2[:, :, 0].T[0:128]
    wblob[0:MEL, W_QW1 : W_QW1 + 480] = qw1.transpose(1, 2, 0).reshape(MEL, 480)
    wblob[0:MEL, W_QW3 : W_QW3 + 80] = qw3[:, :, 0].T
    wblob[0:32, W_QW2B : W_QW2B + 80] = qw2[:, :, 0].T[128:160]
    wblob = wblob.astype(BF16)

    fblob = np.zeros((128, F_COLS), F32)
    fblob[:, F_KB1 : F_KB1 + 8] = (WS * kb1).reshape(8, 128).T
    fblob[:, F_QB1A] = qb1[0:128]
    fblob[0:32, F_QB1B] = qb1[128:160]
    fblob[0:MEL, F_QB2] = qb2
    fblob[0:MEL, F_QB3S] = 2.0 * TEMP * qb3
    fblob[0:ATT, F_KB2] = kb2

    # log-prior and masked prior, chunk-major [row-in-chunk, {lp,pm}, chunk, T2]
    lp = np.log(prior + 1e-8)                                   # (B, 1000, 256)
    pmm = (prior + 1e-8) * (1.0 - mask[:, :, 0].astype(F32))[:, None, :]
    lppm = np.zeros((B, 2, 1024, T2), F32)
    lppm[:, 0, :T1] = lp
    lppm[:, 1, :T1] = pmm
    # -> (B, 128, 2, 8, T2)
    lppm = np.ascontiguousarray(
        lppm.reshape(B, 2, 8, 128, T2).transpose(0, 3, 1, 2, 4)
    ).astype(BF16)

    shared = {
        "wblob": wblob,
        "fblob": np.ascontiguousarray(fblob),
        "kw1": kw1_dr,
    }

    in_maps = []
    for b in range(B):
        m = dict(shared)
        m["queries"] = np.ascontiguousarray(qpad[b])
        m["keys"] = np.ascontiguousarray(keys_t[b])
        m["lppm"] = np.ascontiguousarray(lppm[b])
        in_maps.append(m)
    return in_maps


def kernel(**inputs):
    from concourse.bass_utils import run_bass_kernel_spmd

    nc = _get_nc()
    in_maps = _prep_in_maps(inputs)
    res = run_bass_kernel_spmd(nc, in_maps, core_ids=list(range(N_CORES)))
    attn = np.empty((B, 1, T1, T2), F32)
    logp = np.empty((B, 1, T1, T2), F32)
    for i in range(N_CORES):
        o = np.asarray(res.results[i]["out"]).astype(F32)      # [128, 8, 2, T2]
        o = o.transpose(1, 0, 2, 3).reshape(1024, 2, T2)[:T1]  # [1000, 2, T2]
        logp[i, 0] = o[:, 0]
        attn[i, 0] = o[:, 1]
    return attn, logp
